# revision 23
# baseline (speedup 1.0000x reference)
"""Multi-head attention block (B=2, N=2048, C=1024, H=16, D=64) on 8 TRN2 cores.

Sharding: core c -> batch b = c // 4, head-group hg = c % 4 (4 heads per core).
All matmuls in bf16 (1 cyc/row at any moving size in the cost model).
Per core:
  qkT  = Wqk_hg @ x_b^T          (f-tiles [2 heads x 64d, 2048 tok], bf16)
  V    = x_b @ Wv_hg^T           ([tok, 4h x 64d] natural layout -> vaug [V|1])
  S^T  = kT^T q-chunks           (K=64, two heads row-packed; [128 key, 512 q])
  P^T  = exp(S^T)                (ScalarE, no max-subtraction: scores ~ N(0,1))
  O|Z  = P^T-tile^T @ [V|1]      (q-major: out [128 q, 65]; V is the moving
                                  operand -> 65 rows/pass instead of 512)
  O_n  = O * (1/Z)               (DVE reciprocal + per-partition tensor_scalar)
  O^T  = DMA-transpose(O_n)      (XBAR, SBUF->SBUF, 2-byte)
  y    = O^T^T @ Wp^T            (K=128: two heads stacked per pass)
Host sums the 4 head-group partials per batch and adds bias.

Issue order is software-pipelined: S-matmul groups for block k+1 are
interleaved with PV/normalize/proj work of block k-2 so the ScalarE exp
stream (the bottleneck engine, ~133us) never starves.
"""

import numpy as np

import concourse.bass as bass
import concourse.tile as tile
from concourse import bacc, mybir

F32 = mybir.dt.float32
BF16 = mybir.dt.bfloat16
EXP = mybir.ActivationFunctionType.Exp

B, S, C = 2, 2048, 1024
H, D = 16, 64
HPC = 4            # heads per core
NCT = C // 128     # 8 contraction tiles
MT = S // 128      # 16 key tiles
NCH = S // 512     # 4 query chunks


def build_bass(loop_n=None):
    nc = bacc.Bacc("TRN2", target_bir_lowering=False)

    xt_d = nc.dram_tensor("xt", [C, S], BF16, kind="ExternalInput")
    wqk_d = nc.dram_tensor("wqk", [C, 512], BF16, kind="ExternalInput")
    wv_d = nc.dram_tensor("wv", [C, 256], BF16, kind="ExternalInput")
    wp_d = nc.dram_tensor("wp", [256, C], BF16, kind="ExternalInput")
    y_d = nc.dram_tensor("y", [S, C], BF16, kind="ExternalOutput")

    def mm(out, lhsT, rhs, start, stop):
        nc.tensor.matmul(out, lhsT, rhs, start=start, stop=stop)

    with tile.TileContext(nc) as tc:
        with (
            tc.tile_pool(name="persist", bufs=1) as persist,
            tc.tile_pool(name="pt", bufs=6) as pt_pool,
            tc.tile_pool(name="rz", bufs=4) as rz_pool,
            tc.tile_pool(name="osb", bufs=8) as osb_pool,
            tc.tile_pool(name="ysb", bufs=4) as ysb_pool,
            tc.tile_pool(name="ps_s", bufs=2, space="PSUM") as ps_s,
            tc.tile_pool(name="ps_w", bufs=4, space="PSUM") as ps_w,
        ):
            # ---- persistent SBUF tiles ----
            # qkT f-tiles: 0=q_h0|q_h1, 1=q_h2|q_h3, 2=k_h0|k_h1, 3=k_h2|k_h3
            qk_sb = persist.tile([128, 4 * S], BF16, tag="qk")
            # V augmented per key tile m: [v_h0|1|v_h1|1|v_h2|1|v_h3|1]
            vaug = persist.tile([128, MT * 260], BF16, tag="vaug")
            wp_sb = persist.tile([128, 2 * C], BF16, tag="wp")
            xt_sb = persist.tile([128, NCT * S], BF16, tag="xt")
            wqk_sb = persist.tile([128, NCT * 512], BF16, tag="wqk")
            wv_sb = persist.tile([128, NCT * 256], BF16, tag="wv")
            # normalized-transposed O per (pair, ch): [128 = 2 heads x 64d, 512 q]
            onorm = [
                persist.tile([64, 0], BF16, tag="dummy")  # placeholder, replaced below
                for _ in range(0)
            ]
            onorm = {}
            for p in range(2):
                for c in range(NCH):
                    onorm[(p, c)] = persist.tile(
                        [128, 512], BF16, tag=f"onorm{p}{c}", name=f"onorm{p}{c}"
                    )

            nc.vector.memset(vaug, 1.0)

            def dma_wqk_f(f):
                # wqk_sb layout: [128, f*1024 + ct*128 + j]
                nc.sync.dma_start(
                    out=wqk_sb[:, f * 1024 : (f + 1) * 1024].rearrange(
                        "p (c j) -> p c j", c=NCT
                    ),
                    in_=wqk_d[:, f * 128 : (f + 1) * 128].rearrange(
                        "(c p) j -> p c j", p=128
                    ),
                )

            # DMA priority order: weights the first S block needs, then x,
            # then V weights (window-0 fillers), then the pair-1 weights.
            dma_wqk_f(2)
            dma_wqk_f(0)
            for ct in range(NCT):
                nc.sync.dma_start(
                    out=xt_sb[:, ct * S : (ct + 1) * S],
                    in_=xt_d[ct * 128 : (ct + 1) * 128, :],
                )
            for ct in range(NCT):
                nc.sync.dma_start(
                    out=wv_sb[:, ct * 256 : (ct + 1) * 256],
                    in_=wv_d[ct * 128 : (ct + 1) * 128, :],
                )
            for f in (3, 1):
                dma_wqk_f(f)
            for p in range(2):
                nc.sync.dma_start(
                    out=wp_sb[:, p * C : (p + 1) * C],
                    in_=wp_d[p * 128 : (p + 1) * 128, :],
                )

            # ---------------- step generators ----------------

            def qk_f_block(chains, ps_a):
                """qkT chains [(f, sc), ...] ct-outer concurrently (needs
                len(chains) free ps_a bufs). Used during the input DMA
                stream so PE accumulates as each x chunk lands."""
                qps = {
                    fs: ps_a.tile([128, 512], F32, tag="psw",
                                  name=f"qps{fs[0]}_{fs[1]}")
                    for fs in chains
                }
                for ct in range(NCT):
                    for f, sc in chains:
                        mm(
                            qps[(f, sc)],
                            wqk_sb[:, f * 1024 + ct * 128 : f * 1024 + (ct + 1) * 128],
                            xt_sb[:, ct * S + sc * 512 : ct * S + (sc + 1) * 512],
                            start=(ct == 0),
                            stop=(ct == NCT - 1),
                        )
                for f, sc in chains:
                    nc.vector.tensor_copy(
                        qk_sb[:, f * S + sc * 512 : f * S + (sc + 1) * 512],
                        qps[(f, sc)],
                    )

            def qk_f_steps(f, ps_a, scs=range(4)):
                """Half-sc ct-inner steps (~0.85us each) for filler use once
                the input DMAs have landed."""
                for sc in scs:
                    qps_box = []
                    def step1(f=f, sc=sc, qps_box=qps_box):
                        qps = ps_a.tile([128, 512], F32, tag="psw",
                                        name=f"qps{f}_{sc}")
                        qps_box.append(qps)
                        for ct in range(NCT // 2):
                            mm(
                                qps,
                                wqk_sb[:, f * 1024 + ct * 128 : f * 1024 + (ct + 1) * 128],
                                xt_sb[:, ct * S + sc * 512 : ct * S + (sc + 1) * 512],
                                start=(ct == 0),
                                stop=False,
                            )
                    def step2(f=f, sc=sc, qps_box=qps_box):
                        qps = qps_box[0]
                        for ct in range(NCT // 2, NCT):
                            mm(
                                qps,
                                wqk_sb[:, f * 1024 + ct * 128 : f * 1024 + (ct + 1) * 128],
                                xt_sb[:, ct * S + sc * 512 : ct * S + (sc + 1) * 512],
                                start=False,
                                stop=(ct == NCT - 1),
                            )
                        nc.vector.tensor_copy(
                            qk_sb[:, f * S + sc * 512 : f * S + (sc + 1) * 512],
                            qps,
                        )
                    yield step1
                    yield step2

            def v_steps(m0, m1, pool, tag):
                """V key-tiles m0..m1-1 -> vaug (strided, ones preserved)."""
                for m in range(m0, m1):
                    def step(m=m, pool=pool, tag=tag):
                        vps_full = pool.tile([128, 512], F32, tag=tag,
                                             name=f"vps{m}")
                        vps = vps_full[:, 0:256]
                        for ct in range(NCT):
                            mm(
                                vps,
                                xt_sb[:, ct * S + m * 128 : ct * S + (m + 1) * 128],
                                wv_sb[:, ct * 256 : (ct + 1) * 256],
                                start=(ct == 0),
                                stop=(ct == NCT - 1),
                            )
                        nc.vector.tensor_copy(
                            vaug[:, m * 260 : (m + 1) * 260].rearrange(
                                "p (h c) -> p h c", c=65
                            )[:, :, 0:64],
                            vps.rearrange("p (h c) -> p h c", c=64),
                        )
                    yield step

            def s_steps(pair, ch, pts, head_major=False):
                """Score+exp stream for block (pair, ch). Default: 8 mg
                steps, each head A then head B over key tiles 2mg, 2mg+1.
                head_major: 16 single-head steps, all of head A first (lets
                the tail start head-A PV while head-B exps still run)."""
                qf, kf = pair, 2 + pair
                ptA = pt_pool.tile([128, MT * 512], BF16, tag="pt",
                                   name=f"ptA_{pair}_{ch}")
                ptB = pt_pool.tile([128, MT * 512], BF16, tag="pt",
                                   name=f"ptB_{pair}_{ch}")
                pts[(pair, ch)] = (ptA, ptB)

                def sub(mg, hh, pt):
                    lo = 64 * hh
                    sps = ps_s.tile([128, 1024], F32, tag="sps",
                                    name=f"s{pair}{ch}{mg}{hh}")
                    for j in range(2):
                        m = 2 * mg + j
                        mm(
                            sps[:, j * 512 : (j + 1) * 512],
                            qk_sb[lo:lo + 64, kf * S + m * 128 : kf * S + (m + 1) * 128],
                            qk_sb[lo:lo + 64, qf * S + ch * 512 : qf * S + (ch + 1) * 512],
                            start=True,
                            stop=True,
                        )
                    nc.scalar.activation(
                        pt[:, 2 * mg * 512 : (2 * mg + 2) * 512], sps, EXP
                    )

                if head_major:
                    for hh, pt in ((0, ptA), (1, ptB)):
                        for mg in range(MT // 2):
                            def step(mg=mg, hh=hh, pt=pt):
                                sub(mg, hh, pt)
                            yield step
                else:
                    for mg in range(MT // 2):
                        def step(mg=mg, ptA=ptA, ptB=ptB):
                            sub(mg, 0, ptA)
                            sub(mg, 1, ptB)
                        yield step

            osbs = {}

            def work_steps(pair, ch, pts, ps_w, phases="AB", y_eng=None):
                """PV + normalize + transpose for block (pair, ch); when
                pair==1 also project+store. phases: "AB" both heads,
                "A"/"B" only that head (B also transposes/projects).
                y_eng: engine for the y PSUM->SBUF copy (default DVE)."""
                ptA, ptB = pts[(pair, ch)]
                for qt in range(4):
                    for hh, pt in ((0, ptA), (1, ptB)):
                        if "AB"[hh] not in phases:
                            continue
                        def hstep(pair=pair, ch=ch, qt=qt, hh=hh, pt=pt):
                            if hh == 0:
                                osbs[(pair, ch, qt)] = osb_pool.tile(
                                    [128, 128], BF16, tag="osb",
                                    name=f"osb{pair}{ch}{qt}")
                            osb = osbs[(pair, ch, qt)]
                            h = 2 * pair + hh
                            ops = ps_w.tile([128, 512], F32, tag="psw",
                                            name=f"o{pair}{ch}{qt}{hh}")
                            for m in range(MT):
                                mm(
                                    ops[:, 0:65],
                                    pt[:, m * 512 + qt * 128 : m * 512 + qt * 128 + 128],
                                    vaug[:, m * 260 + h * 65 : m * 260 + h * 65 + 65],
                                    start=(m == 0),
                                    stop=(m == MT - 1),
                                )
                            rz = rz_pool.tile([128, 1], F32, tag="rz",
                                              name=f"rz{pair}{ch}{qt}{hh}")
                            nc.vector.reciprocal(out=rz, in_=ops[:, 64:65])
                            nc.vector.tensor_scalar_mul(
                                osb[:, hh * 64 : hh * 64 + 64],
                                ops[:, 0:64],
                                rz,
                            )
                            if hh == 1:
                                # full [128 q, 128 d-pair] -> [128 d-pair, 128 q]
                                nc.sync.dma_start(
                                    out=onorm[(pair, ch)][:, qt * 128 : (qt + 1) * 128],
                                    in_=osb,
                                    transpose=True,
                                )
                        yield hstep
                    if pair == 1 and "B" in phases:
                        for fc in range(2):
                            def pstep(ch=ch, qt=qt, fc=fc):
                                st = ch * 4 + qt
                                ysb = ysb_pool.tile([128, 512], BF16,
                                                    tag="ysb",
                                                    name=f"ysb{ch}{qt}{fc}")
                                yps = ps_w.tile([128, 512], F32, tag="psw",
                                                name=f"y{ch}{qt}{fc}")
                                for p in range(2):
                                    mm(
                                        yps,
                                        onorm[(p, ch)][:, qt * 128 : (qt + 1) * 128],
                                        wp_sb[:, p * C + fc * 512 : p * C + (fc + 1) * 512],
                                        start=(p == 0),
                                        stop=(p == 1),
                                    )
                                if y_eng is None:
                                    nc.vector.tensor_copy(ysb, yps)
                                else:
                                    y_eng.copy(ysb, yps)
                                nc.sync.dma_start(
                                    out=y_d[st * 128 : (st + 1) * 128,
                                            fc * 512 : (fc + 1) * 512],
                                    in_=ysb,
                                )
                            yield pstep

            def chain(*gens):
                for g in gens:
                    yield from g

            def interleave(sgen, fgen):
                """Alternate one S step with one filler step; drain leftovers."""
                while True:
                    s = next(sgen, None)
                    if s is not None:
                        s()
                    f = next(fgen, None)
                    if f is not None:
                        f()
                    if s is None and f is None:
                        return

            def drain(g):
                for step in g:
                    step()

            # ---------------- pipelined schedule ----------------
            blocks = [(p, c) for p in range(2) for c in range(NCH)]
            pts = {}

            # during the input DMA stream: the columns S(p0,c0) needs first
            qk_f_block([(2, 0), (0, 0), (2, 1), (2, 2)], ps_w)
            # window 0: S(p0,c0) || k0 tail + q0 rest + V m0..4
            interleave(
                s_steps(0, 0, pts),
                chain(
                    qk_f_steps(2, ps_w, scs=(3,)),
                    qk_f_steps(0, ps_w, scs=(1, 2, 3)),
                    v_steps(0, 5, ps_w, 'psw'),
                ),
            )
            # window 1: S(p0,c1) || V m5..15
            interleave(
                s_steps(0, 1, pts),
                v_steps(5, MT, ps_w, 'psw'),
            )
            # window 2: S(p0,c2) || k1 weights + work(B0)
            interleave(
                s_steps(0, 2, pts),
                chain(qk_f_steps(3, ps_w),
                      work_steps(*blocks[0], pts, ps_w)),
            )
            # window 3: S(p0,c3) || q1 weights + work(B1)
            interleave(
                s_steps(0, 3, pts),
                chain(qk_f_steps(1, ps_w),
                      work_steps(*blocks[1], pts, ps_w)),
            )
            # windows 4..6: S(block k) || work(block k-2)
            for k in range(4, len(blocks) - 1):
                interleave(
                    s_steps(*blocks[k], pts),
                    work_steps(*blocks[k - 2], pts, ps_w),
                )
            # window 7: S(B7) || work(B5) then work(B6) (exp(B6) is already
            # done by the time PE reaches it -- ScalarE runs a block behind)
            interleave(
                s_steps(*blocks[7], pts),
                chain(work_steps(*blocks[5], pts, ps_w),
                      work_steps(*blocks[6], pts, ps_w, y_eng=nc.scalar)),
            )
            # tail: only the last block's PV/norm/proj remains; ScalarE is
            # idle after the final exp so it takes the y copies
            drain(work_steps(*blocks[7], pts, ps_w, y_eng=nc.scalar))

    nc.compile()
    return nc


def make_core_inputs(x, Wqkv, Wproj):
    """Per-core input dicts. Core c: batch c//4, heads 4*(c%4) .. 4*(c%4)+3."""
    bf16 = mybir.dt.np(BF16)
    scale = D**-0.5  # 1/8, exact in bf16
    xts = [np.ascontiguousarray(x[b].T).astype(bf16) for b in range(B)]
    in_maps = []
    for core in range(8):
        b, hg = core // 4, core % 4
        heads = [HPC * hg + i for i in range(HPC)]
        rows_q = np.concatenate([Wqkv[D * h : D * (h + 1)] for h in heads]) * scale
        rows_k = np.concatenate([Wqkv[C + D * h : C + D * (h + 1)] for h in heads])
        wqk = np.ascontiguousarray(np.concatenate([rows_q, rows_k]).T).astype(bf16)
        wv = np.ascontiguousarray(
            np.concatenate([Wqkv[2 * C + D * h : 2 * C + D * (h + 1)] for h in heads]).T
        ).astype(bf16)
        wp = np.ascontiguousarray(
            np.concatenate([Wproj[:, D * h : D * (h + 1)] for h in heads], axis=1).T
        ).astype(bf16)
        in_maps.append({"xt": xts[b], "wqk": wqk, "wv": wv, "wp": wp})
    return in_maps


_EXEC_CACHE = {}


def _get_executor():
    """Build + jit the 8-core SPMD executable once per process."""
    if "fn" in _EXEC_CACHE:
        return _EXEC_CACHE
    import jax
    from jax.sharding import Mesh, PartitionSpec
    from jax.experimental.shard_map import shard_map
    from concourse import bass2jax
    from concourse.bass2jax import _bass_exec_p, partition_id_tensor

    nc = build_bass()
    bass2jax.install_neuronx_cc_hook()
    pid = nc.partition_id_tensor.name if nc.partition_id_tensor else None
    in_names, out_names, out_avals = [], [], []
    for alloc in nc.m.functions[0].allocations:
        if not isinstance(alloc, mybir.MemoryLocationSet):
            continue
        name = alloc.memorylocations[0].name
        if alloc.kind == "ExternalInput":
            if name != pid:
                in_names.append(name)
        elif alloc.kind == "ExternalOutput":
            out_names.append(name)
            out_avals.append(
                jax.core.ShapedArray(
                    tuple(alloc.tensor_shape), mybir.dt.np(alloc.dtype)
                )
            )
    n_params = len(in_names)
    all_names = list(in_names) + list(out_names) + ([pid] if pid else [])

    def body(*args):
        *ins, yb = args
        operands = list(ins) + [yb]
        if pid:
            operands.append(partition_id_tensor())
        outs = _bass_exec_p.bind(
            *operands,
            out_avals=tuple(out_avals),
            in_names=tuple(all_names),
            out_names=tuple(out_names),
            lowering_input_output_aliases=(),
            sim_require_finite=True,
            sim_require_nnan=True,
            nc=nc,
        )
        return outs[0]

    mesh = Mesh(np.asarray(jax.devices()[:8]), ("core",))
    fn = jax.jit(
        shard_map(
            body,
            mesh=mesh,
            in_specs=(PartitionSpec("core"),) * (n_params + 1),
            out_specs=PartitionSpec("core"),
            check_rep=False,
        ),
        donate_argnums=(n_params,),
    )
    _EXEC_CACHE.update(fn=fn, in_names=in_names)
    return _EXEC_CACHE


def kernel(x, Wqkv, Wproj, bproj):
    x = np.asarray(x, dtype=np.float32)
    Wqkv = np.asarray(Wqkv, dtype=np.float32)
    Wproj = np.asarray(Wproj, dtype=np.float32)
    bproj = np.asarray(bproj, dtype=np.float32)

    ex = _get_executor()
    in_maps = make_core_inputs(x, Wqkv, Wproj)
    glob_ins = [
        np.concatenate([np.asarray(m[name]) for m in in_maps], axis=0)
        for name in ex["in_names"]
    ]
    y0 = np.zeros((8 * S, C), mybir.dt.np(BF16))
    out = np.asarray(ex["fn"](*glob_ins, y0))  # [8*S, C] bf16 partials

    y = np.zeros((B, S, C), dtype=np.float32)
    for core in range(8):
        y[core // 4] += np.asarray(out[core * S : (core + 1) * S, :], np.float32)
    y += bproj
    return y


# revision 24
# speedup vs baseline: 1.0330x; 1.0330x over previous
"""Multi-head attention block (B=2, N=2048, C=1024, H=16, D=64) on 8 TRN2 cores.

Sharding: core c -> batch b = c // 4, head-group hg = c % 4 (4 heads per core).
All matmuls in bf16 (1 cyc/row at any moving size in the cost model).
Per core:
  qkT  = Wqk_hg @ x_b^T          (f-tiles [2 heads x 64d, 2048 tok], bf16)
  V    = x_b @ Wv_hg^T           ([tok, 4h x 64d] natural layout -> vaug [V|1])
  S^T  = kT^T q-chunks           (K=64, two heads row-packed; [128 key, 512 q])
  P^T  = exp(S^T)                (ScalarE, no max-subtraction: scores ~ N(0,1))
  O|Z  = P^T-tile^T @ [V|1]      (q-major: out [128 q, 65]; V is the moving
                                  operand -> 65 rows/pass instead of 512)
  O_n  = O * (1/Z)               (DVE reciprocal + per-partition tensor_scalar)
  O^T  = DMA-transpose(O_n)      (XBAR, SBUF->SBUF, 2-byte)
  y    = O^T^T @ Wp^T            (K=128: two heads stacked per pass)
Host sums the 4 head-group partials per batch and adds bias.

Issue order is software-pipelined: S-matmul groups for block k+1 are
interleaved with PV/normalize/proj work of block k-2 so the ScalarE exp
stream (the bottleneck engine, ~133us) never starves.
"""

import numpy as np

import concourse.bass as bass
import concourse.tile as tile
from concourse import bacc, mybir

F32 = mybir.dt.float32
BF16 = mybir.dt.bfloat16
EXP = mybir.ActivationFunctionType.Exp

B, S, C = 2, 2048, 1024
H, D = 16, 64
HPC = 4            # heads per core
NCT = C // 128     # 8 contraction tiles
MT = S // 128      # 16 key tiles
NCH = S // 512     # 4 query chunks


def build_bass(loop_n=None):
    nc = bacc.Bacc("TRN2", target_bir_lowering=False)

    xt_d = nc.dram_tensor("xt", [C, S], BF16, kind="ExternalInput")
    wqk_d = nc.dram_tensor("wqk", [C, 512], BF16, kind="ExternalInput")
    wv_d = nc.dram_tensor("wv", [C, 256], BF16, kind="ExternalInput")
    wp_d = nc.dram_tensor("wp", [256, C], BF16, kind="ExternalInput")
    y_d = nc.dram_tensor("y", [S, C], BF16, kind="ExternalOutput")

    def mm(out, lhsT, rhs, start, stop):
        nc.tensor.matmul(out, lhsT, rhs, start=start, stop=stop)

    with tile.TileContext(nc) as tc:
        with (
            tc.tile_pool(name="persist", bufs=1) as persist,
            tc.tile_pool(name="pt", bufs=6) as pt_pool,
            tc.tile_pool(name="rz", bufs=4) as rz_pool,
            tc.tile_pool(name="osb", bufs=8) as osb_pool,
            tc.tile_pool(name="ysb", bufs=4) as ysb_pool,
            tc.tile_pool(name="ps_s", bufs=2, space="PSUM") as ps_s,
            tc.tile_pool(name="ps_w", bufs=4, space="PSUM") as ps_w,
        ):
            # ---- persistent SBUF tiles ----
            # qkT f-tiles: 0=q_h0|q_h1, 1=q_h2|q_h3, 2=k_h0|k_h1, 3=k_h2|k_h3
            qk_sb = persist.tile([128, 4 * S], BF16, tag="qk")
            # V augmented per key tile m: [v_h0|1|v_h1|1|v_h2|1|v_h3|1]
            vaug = persist.tile([128, MT * 260], BF16, tag="vaug")
            wp_sb = persist.tile([128, 2 * C], BF16, tag="wp")
            xt_sb = persist.tile([128, NCT * S], BF16, tag="xt")
            wqk_sb = persist.tile([128, NCT * 512], BF16, tag="wqk")
            wv_sb = persist.tile([128, NCT * 256], BF16, tag="wv")
            # normalized-transposed O per (pair, ch): [128 = 2 heads x 64d, 512 q]
            onorm = [
                persist.tile([64, 0], BF16, tag="dummy")  # placeholder, replaced below
                for _ in range(0)
            ]
            onorm = {}
            for p in range(2):
                for c in range(NCH):
                    onorm[(p, c)] = persist.tile(
                        [128, 512], BF16, tag=f"onorm{p}{c}", name=f"onorm{p}{c}"
                    )

            nc.vector.memset(vaug, 1.0)

            def dma_wqk_f(f):
                # wqk_sb layout: [128, f*1024 + ct*128 + j]
                nc.sync.dma_start(
                    out=wqk_sb[:, f * 1024 : (f + 1) * 1024].rearrange(
                        "p (c j) -> p c j", c=NCT
                    ),
                    in_=wqk_d[:, f * 128 : (f + 1) * 128].rearrange(
                        "(c p) j -> p c j", p=128
                    ),
                )

            # DMA priority order: weights the first S block needs, then x,
            # then V weights (window-0 fillers), then the pair-1 weights.
            dma_wqk_f(2)
            dma_wqk_f(0)
            for ct in range(NCT):
                nc.sync.dma_start(
                    out=xt_sb[:, ct * S : (ct + 1) * S],
                    in_=xt_d[ct * 128 : (ct + 1) * 128, :],
                )
            for ct in range(NCT):
                nc.sync.dma_start(
                    out=wv_sb[:, ct * 256 : (ct + 1) * 256],
                    in_=wv_d[ct * 128 : (ct + 1) * 128, :],
                )
            for f in (3, 1):
                dma_wqk_f(f)
            for p in range(2):
                nc.sync.dma_start(
                    out=wp_sb[:, p * C : (p + 1) * C],
                    in_=wp_d[p * 128 : (p + 1) * 128, :],
                )

            # ---------------- step generators ----------------

            def qk_f_block(chains, ps_a):
                """qkT chains [(f, sc), ...] ct-outer concurrently (needs
                len(chains) free ps_a bufs). Used during the input DMA
                stream so PE accumulates as each x chunk lands."""
                qps = {
                    fs: ps_a.tile([128, 512], F32, tag="psw",
                                  name=f"qps{fs[0]}_{fs[1]}")
                    for fs in chains
                }
                for ct in range(NCT):
                    for f, sc in chains:
                        mm(
                            qps[(f, sc)],
                            wqk_sb[:, f * 1024 + ct * 128 : f * 1024 + (ct + 1) * 128],
                            xt_sb[:, ct * S + sc * 512 : ct * S + (sc + 1) * 512],
                            start=(ct == 0),
                            stop=(ct == NCT - 1),
                        )
                for f, sc in chains:
                    nc.vector.tensor_copy(
                        qk_sb[:, f * S + sc * 512 : f * S + (sc + 1) * 512],
                        qps[(f, sc)],
                    )

            def qk_f_steps(f, ps_a, scs=range(4)):
                """Half-sc ct-inner steps (~0.85us each) for filler use once
                the input DMAs have landed."""
                for sc in scs:
                    qps_box = []
                    def step1(f=f, sc=sc, qps_box=qps_box):
                        qps = ps_a.tile([128, 512], F32, tag="psw",
                                        name=f"qps{f}_{sc}")
                        qps_box.append(qps)
                        for ct in range(NCT // 2):
                            mm(
                                qps,
                                wqk_sb[:, f * 1024 + ct * 128 : f * 1024 + (ct + 1) * 128],
                                xt_sb[:, ct * S + sc * 512 : ct * S + (sc + 1) * 512],
                                start=(ct == 0),
                                stop=False,
                            )
                    def step2(f=f, sc=sc, qps_box=qps_box):
                        qps = qps_box[0]
                        for ct in range(NCT // 2, NCT):
                            mm(
                                qps,
                                wqk_sb[:, f * 1024 + ct * 128 : f * 1024 + (ct + 1) * 128],
                                xt_sb[:, ct * S + sc * 512 : ct * S + (sc + 1) * 512],
                                start=False,
                                stop=(ct == NCT - 1),
                            )
                        nc.vector.tensor_copy(
                            qk_sb[:, f * S + sc * 512 : f * S + (sc + 1) * 512],
                            qps,
                        )
                    yield step1
                    yield step2

            def v_steps(m0, m1, pool, tag):
                """V key-tiles m0..m1-1 -> vaug (strided, ones preserved)."""
                for m in range(m0, m1):
                    def step(m=m, pool=pool, tag=tag):
                        vps_full = pool.tile([128, 512], F32, tag=tag,
                                             name=f"vps{m}")
                        vps = vps_full[:, 0:256]
                        for ct in range(NCT):
                            mm(
                                vps,
                                xt_sb[:, ct * S + m * 128 : ct * S + (m + 1) * 128],
                                wv_sb[:, ct * 256 : (ct + 1) * 256],
                                start=(ct == 0),
                                stop=(ct == NCT - 1),
                            )
                        nc.vector.tensor_copy(
                            vaug[:, m * 260 : (m + 1) * 260].rearrange(
                                "p (h c) -> p h c", c=65
                            )[:, :, 0:64],
                            vps.rearrange("p (h c) -> p h c", c=64),
                        )
                    yield step

            def s_steps(pair, ch, pts, head_major=False):
                """Score+exp stream for block (pair, ch). Default: 8 mg
                steps, each head A then head B over key tiles 2mg, 2mg+1.
                head_major: 16 single-head steps, all of head A first (lets
                the tail start head-A PV while head-B exps still run)."""
                qf, kf = pair, 2 + pair
                ptA = pt_pool.tile([128, MT * 512], BF16, tag="pt",
                                   name=f"ptA_{pair}_{ch}")
                ptB = pt_pool.tile([128, MT * 512], BF16, tag="pt",
                                   name=f"ptB_{pair}_{ch}")
                pts[(pair, ch)] = (ptA, ptB)

                def sub(mg, hh, pt):
                    lo = 64 * hh
                    sps = ps_s.tile([128, 1024], F32, tag="sps",
                                    name=f"s{pair}{ch}{mg}{hh}")
                    for j in range(2):
                        m = 2 * mg + j
                        mm(
                            sps[:, j * 512 : (j + 1) * 512],
                            qk_sb[lo:lo + 64, kf * S + m * 128 : kf * S + (m + 1) * 128],
                            qk_sb[lo:lo + 64, qf * S + ch * 512 : qf * S + (ch + 1) * 512],
                            start=True,
                            stop=True,
                        )
                    nc.scalar.activation(
                        pt[:, 2 * mg * 512 : (2 * mg + 2) * 512], sps, EXP
                    )

                if head_major:
                    for hh, pt in ((0, ptA), (1, ptB)):
                        for mg in range(MT // 2):
                            def step(mg=mg, hh=hh, pt=pt):
                                sub(mg, hh, pt)
                            yield step
                else:
                    for mg in range(MT // 2):
                        def step(mg=mg, ptA=ptA, ptB=ptB):
                            sub(mg, 0, ptA)
                            sub(mg, 1, ptB)
                        yield step

            osbs = {}

            def work_steps(pair, ch, pts, ps_w, phases="AB", y_eng=None):
                """PV + normalize + transpose for block (pair, ch); when
                pair==1 also project+store. phases: "AB" both heads,
                "A"/"B" only that head (B also transposes/projects).
                y_eng: engine for the y PSUM->SBUF copy (default DVE)."""
                ptA, ptB = pts[(pair, ch)]
                for qt in range(4):
                    for hh, pt in ((0, ptA), (1, ptB)):
                        if "AB"[hh] not in phases:
                            continue
                        def hstep(pair=pair, ch=ch, qt=qt, hh=hh, pt=pt):
                            if hh == 0:
                                osbs[(pair, ch, qt)] = osb_pool.tile(
                                    [128, 128], BF16, tag="osb",
                                    name=f"osb{pair}{ch}{qt}")
                            osb = osbs[(pair, ch, qt)]
                            h = 2 * pair + hh
                            ops = ps_w.tile([128, 512], F32, tag="psw",
                                            name=f"o{pair}{ch}{qt}{hh}")
                            for m in range(MT):
                                mm(
                                    ops[:, 0:65],
                                    pt[:, m * 512 + qt * 128 : m * 512 + qt * 128 + 128],
                                    vaug[:, m * 260 + h * 65 : m * 260 + h * 65 + 65],
                                    start=(m == 0),
                                    stop=(m == MT - 1),
                                )
                            rz = rz_pool.tile([128, 1], F32, tag="rz",
                                              name=f"rz{pair}{ch}{qt}{hh}")
                            nc.vector.reciprocal(out=rz, in_=ops[:, 64:65])
                            nc.vector.tensor_scalar_mul(
                                osb[:, hh * 64 : hh * 64 + 64],
                                ops[:, 0:64],
                                rz,
                            )
                            if hh == 1:
                                # full [128 q, 128 d-pair] -> [128 d-pair, 128 q]
                                nc.sync.dma_start(
                                    out=onorm[(pair, ch)][:, qt * 128 : (qt + 1) * 128],
                                    in_=osb,
                                    transpose=True,
                                )
                        yield hstep
                    if pair == 1 and "B" in phases:
                        for fc in range(2):
                            def pstep(ch=ch, qt=qt, fc=fc):
                                st = ch * 4 + qt
                                ysb = ysb_pool.tile([128, 512], BF16,
                                                    tag="ysb",
                                                    name=f"ysb{ch}{qt}{fc}")
                                yps = ps_w.tile([128, 512], F32, tag="psw",
                                                name=f"y{ch}{qt}{fc}")
                                for p in range(2):
                                    mm(
                                        yps,
                                        onorm[(p, ch)][:, qt * 128 : (qt + 1) * 128],
                                        wp_sb[:, p * C + fc * 512 : p * C + (fc + 1) * 512],
                                        start=(p == 0),
                                        stop=(p == 1),
                                    )
                                if y_eng is None:
                                    nc.vector.tensor_copy(ysb, yps)
                                else:
                                    y_eng.copy(ysb, yps)
                                nc.sync.dma_start(
                                    out=y_d[st * 128 : (st + 1) * 128,
                                            fc * 512 : (fc + 1) * 512],
                                    in_=ysb,
                                )
                            yield pstep

            def chain(*gens):
                for g in gens:
                    yield from g

            def interleave(sgen, fgen):
                """Alternate one S step with one filler step; drain leftovers."""
                while True:
                    s = next(sgen, None)
                    if s is not None:
                        s()
                    f = next(fgen, None)
                    if f is not None:
                        f()
                    if s is None and f is None:
                        return

            def drain(g):
                for step in g:
                    step()

            # ---------------- pipelined schedule ----------------
            blocks = [(p, c) for p in range(2) for c in range(NCH)]
            pts = {}

            # during the input DMA stream: the columns S(p0,c0) needs first
            qk_f_block([(2, 0), (0, 0), (2, 1), (2, 2)], ps_w)
            # window 0: S(p0,c0) || k0 tail + q0 rest + V m0..4
            interleave(
                s_steps(0, 0, pts),
                chain(
                    qk_f_steps(2, ps_w, scs=(3,)),
                    qk_f_steps(0, ps_w, scs=(1, 2, 3)),
                    v_steps(0, 5, ps_w, 'psw'),
                ),
            )
            # window 1: S(p0,c1) || V m5..15
            interleave(
                s_steps(0, 1, pts),
                v_steps(5, MT, ps_w, 'psw'),
            )
            # window 2: S(p0,c2) || k1 weights + work(B0)
            interleave(
                s_steps(0, 2, pts),
                chain(qk_f_steps(3, ps_w),
                      work_steps(*blocks[0], pts, ps_w)),
            )
            # window 3: S(p0,c3) || q1 weights + work(B1)
            interleave(
                s_steps(0, 3, pts),
                chain(qk_f_steps(1, ps_w),
                      work_steps(*blocks[1], pts, ps_w)),
            )
            # windows 4..6: S(block k) || work(block k-2)
            for k in range(4, len(blocks) - 1):
                interleave(
                    s_steps(*blocks[k], pts),
                    work_steps(*blocks[k - 2], pts, ps_w),
                )
            # window 7: S(B7) || work(B5) then work(B6) (exp(B6) is already
            # done by the time PE reaches it -- ScalarE runs a block behind)
            interleave(
                s_steps(*blocks[7], pts),
                chain(work_steps(*blocks[5], pts, ps_w),
                      work_steps(*blocks[6], pts, ps_w)),
            )
            # tail: only the last block's PV/norm/proj remains
            drain(work_steps(*blocks[7], pts, ps_w))

    nc.compile()
    return nc


def make_core_inputs(x, Wqkv, Wproj):
    """Per-core input dicts. Core c: batch c//4, heads 4*(c%4) .. 4*(c%4)+3."""
    bf16 = mybir.dt.np(BF16)
    scale = D**-0.5  # 1/8, exact in bf16
    xts = [np.ascontiguousarray(x[b].T).astype(bf16) for b in range(B)]
    in_maps = []
    for core in range(8):
        b, hg = core // 4, core % 4
        heads = [HPC * hg + i for i in range(HPC)]
        rows_q = np.concatenate([Wqkv[D * h : D * (h + 1)] for h in heads]) * scale
        rows_k = np.concatenate([Wqkv[C + D * h : C + D * (h + 1)] for h in heads])
        wqk = np.ascontiguousarray(np.concatenate([rows_q, rows_k]).T).astype(bf16)
        wv = np.ascontiguousarray(
            np.concatenate([Wqkv[2 * C + D * h : 2 * C + D * (h + 1)] for h in heads]).T
        ).astype(bf16)
        wp = np.ascontiguousarray(
            np.concatenate([Wproj[:, D * h : D * (h + 1)] for h in heads], axis=1).T
        ).astype(bf16)
        in_maps.append({"xt": xts[b], "wqk": wqk, "wv": wv, "wp": wp})
    return in_maps


_EXEC_CACHE = {}


def _get_executor():
    """Build + jit the 8-core SPMD executable once per process."""
    if "fn" in _EXEC_CACHE:
        return _EXEC_CACHE
    import jax
    from jax.sharding import Mesh, PartitionSpec
    from jax.experimental.shard_map import shard_map
    from concourse import bass2jax
    from concourse.bass2jax import _bass_exec_p, partition_id_tensor

    nc = build_bass()
    bass2jax.install_neuronx_cc_hook()
    pid = nc.partition_id_tensor.name if nc.partition_id_tensor else None
    in_names, out_names, out_avals = [], [], []
    for alloc in nc.m.functions[0].allocations:
        if not isinstance(alloc, mybir.MemoryLocationSet):
            continue
        name = alloc.memorylocations[0].name
        if alloc.kind == "ExternalInput":
            if name != pid:
                in_names.append(name)
        elif alloc.kind == "ExternalOutput":
            out_names.append(name)
            out_avals.append(
                jax.core.ShapedArray(
                    tuple(alloc.tensor_shape), mybir.dt.np(alloc.dtype)
                )
            )
    n_params = len(in_names)
    all_names = list(in_names) + list(out_names) + ([pid] if pid else [])

    def body(*args):
        *ins, yb = args
        operands = list(ins) + [yb]
        if pid:
            operands.append(partition_id_tensor())
        outs = _bass_exec_p.bind(
            *operands,
            out_avals=tuple(out_avals),
            in_names=tuple(all_names),
            out_names=tuple(out_names),
            lowering_input_output_aliases=(),
            sim_require_finite=True,
            sim_require_nnan=True,
            nc=nc,
        )
        return outs[0]

    mesh = Mesh(np.asarray(jax.devices()[:8]), ("core",))
    fn = jax.jit(
        shard_map(
            body,
            mesh=mesh,
            in_specs=(PartitionSpec("core"),) * (n_params + 1),
            out_specs=PartitionSpec("core"),
            check_rep=False,
        ),
        donate_argnums=(n_params,),
    )
    _EXEC_CACHE.update(fn=fn, in_names=in_names)
    return _EXEC_CACHE


def kernel(x, Wqkv, Wproj, bproj):
    x = np.asarray(x, dtype=np.float32)
    Wqkv = np.asarray(Wqkv, dtype=np.float32)
    Wproj = np.asarray(Wproj, dtype=np.float32)
    bproj = np.asarray(bproj, dtype=np.float32)

    ex = _get_executor()
    in_maps = make_core_inputs(x, Wqkv, Wproj)
    glob_ins = [
        np.concatenate([np.asarray(m[name]) for m in in_maps], axis=0)
        for name in ex["in_names"]
    ]
    y0 = np.zeros((8 * S, C), mybir.dt.np(BF16))
    out = np.asarray(ex["fn"](*glob_ins, y0))  # [8*S, C] bf16 partials

    y = np.zeros((B, S, C), dtype=np.float32)
    for core in range(8):
        y[core // 4] += np.asarray(out[core * S : (core + 1) * S, :], np.float32)
    y += bproj
    return y


# revision 25
# speedup vs baseline: 1.0378x; 1.0047x over previous
"""Multi-head attention block (B=2, N=2048, C=1024, H=16, D=64) on 8 TRN2 cores.

Sharding: core c -> batch b = c // 4, head-group hg = c % 4 (4 heads per core).
All matmuls in bf16 (1 cyc/row at any moving size in the cost model).
Per core:
  qkT  = Wqk_hg @ x_b^T          (f-tiles [2 heads x 64d, 2048 tok], bf16)
  V    = x_b @ Wv_hg^T           ([tok, 4h x 64d] natural layout -> vaug [V|1])
  S^T  = kT^T q-chunks           (K=64, two heads row-packed; [128 key, 512 q])
  P^T  = exp(S^T)                (ScalarE, no max-subtraction: scores ~ N(0,1))
  O|Z  = P^T-tile^T @ [V|1]      (q-major: out [128 q, 65]; V is the moving
                                  operand -> 65 rows/pass instead of 512)
  O_n  = O * (1/Z)               (DVE reciprocal + per-partition tensor_scalar)
  O^T  = DMA-transpose(O_n)      (XBAR, SBUF->SBUF, 2-byte)
  y    = O^T^T @ Wp^T            (K=128: two heads stacked per pass)
Host sums the 4 head-group partials per batch and adds bias.

Issue order is software-pipelined: S-matmul groups for block k+1 are
interleaved with PV/normalize/proj work of block k-2 so the ScalarE exp
stream (the bottleneck engine, ~133us) never starves.
"""

import numpy as np

import concourse.bass as bass
import concourse.tile as tile
from concourse import bacc, mybir

F32 = mybir.dt.float32
BF16 = mybir.dt.bfloat16
EXP = mybir.ActivationFunctionType.Exp

B, S, C = 2, 2048, 1024
H, D = 16, 64
HPC = 4            # heads per core
NCT = C // 128     # 8 contraction tiles
MT = S // 128      # 16 key tiles
NCH = S // 512     # 4 query chunks


def build_bass(loop_n=None):
    nc = bacc.Bacc("TRN2", target_bir_lowering=False)

    xt_d = nc.dram_tensor("xt", [C, S], BF16, kind="ExternalInput")
    wqk_d = nc.dram_tensor("wqk", [C, 512], BF16, kind="ExternalInput")
    wv_d = nc.dram_tensor("wv", [C, 256], BF16, kind="ExternalInput")
    wp_d = nc.dram_tensor("wp", [256, C], BF16, kind="ExternalInput")
    y_d = nc.dram_tensor("y", [S, C], BF16, kind="ExternalOutput")

    def mm(out, lhsT, rhs, start, stop):
        nc.tensor.matmul(out, lhsT, rhs, start=start, stop=stop)

    with tile.TileContext(nc) as tc:
        with (
            tc.tile_pool(name="persist", bufs=1) as persist,
            tc.tile_pool(name="pt", bufs=6) as pt_pool,
            tc.tile_pool(name="rz", bufs=4) as rz_pool,
            tc.tile_pool(name="osb", bufs=8) as osb_pool,
            tc.tile_pool(name="ysb", bufs=4) as ysb_pool,
            tc.tile_pool(name="ps_s", bufs=2, space="PSUM") as ps_s,
            tc.tile_pool(name="ps_w", bufs=4, space="PSUM") as ps_w,
        ):
            # ---- persistent SBUF tiles ----
            # qkT f-tiles: 0=q_h0|q_h1, 1=q_h2|q_h3, 2=k_h0|k_h1, 3=k_h2|k_h3
            qk_sb = persist.tile([128, 4 * S], BF16, tag="qk")
            # V augmented per key tile m: [v_h0|1|v_h1|1|v_h2|1|v_h3|1]
            vaug = persist.tile([128, MT * 260], BF16, tag="vaug")
            wp_sb = persist.tile([128, 2 * C], BF16, tag="wp")
            xt_sb = persist.tile([128, NCT * S], BF16, tag="xt")
            wqk_sb = persist.tile([128, NCT * 512], BF16, tag="wqk")
            wv_sb = persist.tile([128, NCT * 256], BF16, tag="wv")
            # normalized-transposed O per (pair, ch): [128 = 2 heads x 64d, 512 q]
            onorm = [
                persist.tile([64, 0], BF16, tag="dummy")  # placeholder, replaced below
                for _ in range(0)
            ]
            onorm = {}
            for p in range(2):
                for c in range(NCH):
                    onorm[(p, c)] = persist.tile(
                        [128, 512], BF16, tag=f"onorm{p}{c}", name=f"onorm{p}{c}"
                    )

            nc.vector.memset(vaug, 1.0)

            def dma_wqk_f(f):
                # wqk_sb layout: [128, f*1024 + ct*128 + j]
                nc.sync.dma_start(
                    out=wqk_sb[:, f * 1024 : (f + 1) * 1024].rearrange(
                        "p (c j) -> p c j", c=NCT
                    ),
                    in_=wqk_d[:, f * 128 : (f + 1) * 128].rearrange(
                        "(c p) j -> p c j", p=128
                    ),
                )

            # DMA priority order: weights the first S block needs, then x,
            # then V weights (window-0 fillers), then the pair-1 weights.
            dma_wqk_f(2)
            dma_wqk_f(0)
            for ct in range(NCT):
                nc.sync.dma_start(
                    out=xt_sb[:, ct * S : (ct + 1) * S],
                    in_=xt_d[ct * 128 : (ct + 1) * 128, :],
                )
            for ct in range(NCT):
                nc.sync.dma_start(
                    out=wv_sb[:, ct * 256 : (ct + 1) * 256],
                    in_=wv_d[ct * 128 : (ct + 1) * 128, :],
                )
            for f in (3, 1):
                dma_wqk_f(f)
            for p in range(2):
                nc.sync.dma_start(
                    out=wp_sb[:, p * C : (p + 1) * C],
                    in_=wp_d[p * 128 : (p + 1) * 128, :],
                )

            # ---------------- step generators ----------------

            def qk_f_block(chains, ps_a):
                """qkT chains [(f, sc), ...] ct-outer concurrently (needs
                len(chains) free ps_a bufs). Used during the input DMA
                stream so PE accumulates as each x chunk lands."""
                qps = {
                    fs: ps_a.tile([128, 512], F32, tag="psw",
                                  name=f"qps{fs[0]}_{fs[1]}")
                    for fs in chains
                }
                for ct in range(NCT):
                    for f, sc in chains:
                        mm(
                            qps[(f, sc)],
                            wqk_sb[:, f * 1024 + ct * 128 : f * 1024 + (ct + 1) * 128],
                            xt_sb[:, ct * S + sc * 512 : ct * S + (sc + 1) * 512],
                            start=(ct == 0),
                            stop=(ct == NCT - 1),
                        )
                for f, sc in chains:
                    nc.vector.tensor_copy(
                        qk_sb[:, f * S + sc * 512 : f * S + (sc + 1) * 512],
                        qps[(f, sc)],
                    )

            def qk_f_steps(f, ps_a, scs=range(4)):
                """Half-sc ct-inner steps (~0.85us each) for filler use once
                the input DMAs have landed."""
                for sc in scs:
                    qps_box = []
                    def step1(f=f, sc=sc, qps_box=qps_box):
                        qps = ps_a.tile([128, 512], F32, tag="psw",
                                        name=f"qps{f}_{sc}")
                        qps_box.append(qps)
                        for ct in range(NCT // 2):
                            mm(
                                qps,
                                wqk_sb[:, f * 1024 + ct * 128 : f * 1024 + (ct + 1) * 128],
                                xt_sb[:, ct * S + sc * 512 : ct * S + (sc + 1) * 512],
                                start=(ct == 0),
                                stop=False,
                            )
                    def step2(f=f, sc=sc, qps_box=qps_box):
                        qps = qps_box[0]
                        for ct in range(NCT // 2, NCT):
                            mm(
                                qps,
                                wqk_sb[:, f * 1024 + ct * 128 : f * 1024 + (ct + 1) * 128],
                                xt_sb[:, ct * S + sc * 512 : ct * S + (sc + 1) * 512],
                                start=False,
                                stop=(ct == NCT - 1),
                            )
                        nc.vector.tensor_copy(
                            qk_sb[:, f * S + sc * 512 : f * S + (sc + 1) * 512],
                            qps,
                        )
                    yield step1
                    yield step2

            def v_steps(m0, m1, pool, tag):
                """V key-tiles m0..m1-1 -> vaug (strided, ones preserved)."""
                for m in range(m0, m1):
                    def step(m=m, pool=pool, tag=tag):
                        vps_full = pool.tile([128, 512], F32, tag=tag,
                                             name=f"vps{m}")
                        vps = vps_full[:, 0:256]
                        for ct in range(NCT):
                            mm(
                                vps,
                                xt_sb[:, ct * S + m * 128 : ct * S + (m + 1) * 128],
                                wv_sb[:, ct * 256 : (ct + 1) * 256],
                                start=(ct == 0),
                                stop=(ct == NCT - 1),
                            )
                        nc.vector.tensor_copy(
                            vaug[:, m * 260 : (m + 1) * 260].rearrange(
                                "p (h c) -> p h c", c=65
                            )[:, :, 0:64],
                            vps.rearrange("p (h c) -> p h c", c=64),
                        )
                    yield step

            def s_steps(pair, ch, pts, head_major=False):
                """Score+exp stream for block (pair, ch). Default: 8 mg
                steps, each head A then head B over key tiles 2mg, 2mg+1.
                head_major: 16 single-head steps, all of head A first (lets
                the tail start head-A PV while head-B exps still run)."""
                qf, kf = pair, 2 + pair
                ptA = pt_pool.tile([128, MT * 512], BF16, tag="pt",
                                   name=f"ptA_{pair}_{ch}")
                ptB = pt_pool.tile([128, MT * 512], BF16, tag="pt",
                                   name=f"ptB_{pair}_{ch}")
                pts[(pair, ch)] = (ptA, ptB)

                def sub(mg, hh, pt):
                    lo = 64 * hh
                    sps = ps_s.tile([128, 1024], F32, tag="sps",
                                    name=f"s{pair}{ch}{mg}{hh}")
                    for j in range(2):
                        m = 2 * mg + j
                        mm(
                            sps[:, j * 512 : (j + 1) * 512],
                            qk_sb[lo:lo + 64, kf * S + m * 128 : kf * S + (m + 1) * 128],
                            qk_sb[lo:lo + 64, qf * S + ch * 512 : qf * S + (ch + 1) * 512],
                            start=True,
                            stop=True,
                        )
                    nc.scalar.activation(
                        pt[:, 2 * mg * 512 : (2 * mg + 2) * 512], sps, EXP
                    )

                if head_major:
                    for hh, pt in ((0, ptA), (1, ptB)):
                        for mg in range(MT // 2):
                            def step(mg=mg, hh=hh, pt=pt):
                                sub(mg, hh, pt)
                            yield step
                else:
                    for mg in range(MT // 2):
                        def step(mg=mg, ptA=ptA, ptB=ptB):
                            sub(mg, 0, ptA)
                            sub(mg, 1, ptB)
                        yield step

            osbs = {}

            def work_steps(pair, ch, pts, ps_w, phases="AB", y_eng=None):
                """PV + normalize + transpose for block (pair, ch); when
                pair==1 also project+store. phases: "AB" both heads,
                "A"/"B" only that head (B also transposes/projects).
                y_eng: engine for the y PSUM->SBUF copy (default DVE)."""
                ptA, ptB = pts[(pair, ch)]
                for qt in range(4):
                    for hh, pt in ((0, ptA), (1, ptB)):
                        if "AB"[hh] not in phases:
                            continue
                        def hstep(pair=pair, ch=ch, qt=qt, hh=hh, pt=pt):
                            if hh == 0:
                                osbs[(pair, ch, qt)] = osb_pool.tile(
                                    [128, 128], BF16, tag="osb",
                                    name=f"osb{pair}{ch}{qt}")
                            osb = osbs[(pair, ch, qt)]
                            h = 2 * pair + hh
                            ops = ps_w.tile([128, 512], F32, tag="psw",
                                            name=f"o{pair}{ch}{qt}{hh}")
                            for m in range(MT):
                                mm(
                                    ops[:, 0:65],
                                    pt[:, m * 512 + qt * 128 : m * 512 + qt * 128 + 128],
                                    vaug[:, m * 260 + h * 65 : m * 260 + h * 65 + 65],
                                    start=(m == 0),
                                    stop=(m == MT - 1),
                                )
                            rz = rz_pool.tile([128, 1], F32, tag="rz",
                                              name=f"rz{pair}{ch}{qt}{hh}")
                            nc.vector.reciprocal(out=rz, in_=ops[:, 64:65])
                            nc.vector.tensor_scalar_mul(
                                osb[:, hh * 64 : hh * 64 + 64],
                                ops[:, 0:64],
                                rz,
                            )
                            if hh == 1:
                                # full [128 q, 128 d-pair] -> [128 d-pair, 128 q]
                                nc.sync.dma_start(
                                    out=onorm[(pair, ch)][:, qt * 128 : (qt + 1) * 128],
                                    in_=osb,
                                    transpose=True,
                                )
                        yield hstep
                    if pair == 1 and "B" in phases:
                        for fc in range(2):
                            def pstep(ch=ch, qt=qt, fc=fc):
                                st = ch * 4 + qt
                                ysb = ysb_pool.tile([128, 512], BF16,
                                                    tag="ysb",
                                                    name=f"ysb{ch}{qt}{fc}")
                                yps = ps_w.tile([128, 512], F32, tag="psw",
                                                name=f"y{ch}{qt}{fc}")
                                for p in range(2):
                                    mm(
                                        yps,
                                        onorm[(p, ch)][:, qt * 128 : (qt + 1) * 128],
                                        wp_sb[:, p * C + fc * 512 : p * C + (fc + 1) * 512],
                                        start=(p == 0),
                                        stop=(p == 1),
                                    )
                                if y_eng is None:
                                    nc.vector.tensor_copy(ysb, yps)
                                else:
                                    y_eng.copy(ysb, yps)
                                nc.sync.dma_start(
                                    out=y_d[st * 128 : (st + 1) * 128,
                                            fc * 512 : (fc + 1) * 512],
                                    in_=ysb,
                                )
                            yield pstep

            def chain(*gens):
                for g in gens:
                    yield from g

            def interleave(sgen, fgen, s_head=0):
                """Alternate one S step with one filler step; drain leftovers.
                s_head: emit this many S steps before the first filler."""
                for _ in range(s_head):
                    s = next(sgen, None)
                    if s is not None:
                        s()
                while True:
                    s = next(sgen, None)
                    if s is not None:
                        s()
                    f = next(fgen, None)
                    if f is not None:
                        f()
                    if s is None and f is None:
                        return

            def drain(g):
                for step in g:
                    step()

            # ---------------- pipelined schedule ----------------
            blocks = [(p, c) for p in range(2) for c in range(NCH)]
            pts = {}

            # during the input DMA stream: the columns S(p0,c0) needs first
            qk_f_block([(2, 0), (0, 0), (2, 1), (2, 2)], ps_w)
            # window 0: S(p0,c0) || k0 tail + q0 rest + V m0..4
            interleave(
                s_steps(0, 0, pts),
                chain(
                    qk_f_steps(2, ps_w, scs=(3,)),
                    qk_f_steps(0, ps_w, scs=(1, 2, 3)),
                    v_steps(0, 5, ps_w, 'psw'),
                ),
                s_head=3,
            )
            # window 1: S(p0,c1) || V m5..15
            interleave(
                s_steps(0, 1, pts),
                v_steps(5, MT, ps_w, 'psw'),
            )
            # window 2: S(p0,c2) || k1 weights + work(B0)
            interleave(
                s_steps(0, 2, pts),
                chain(qk_f_steps(3, ps_w),
                      work_steps(*blocks[0], pts, ps_w)),
            )
            # window 3: S(p0,c3) || q1 weights + work(B1)
            interleave(
                s_steps(0, 3, pts),
                chain(qk_f_steps(1, ps_w),
                      work_steps(*blocks[1], pts, ps_w)),
            )
            # windows 4..6: S(block k) || work(block k-2)
            for k in range(4, len(blocks) - 1):
                interleave(
                    s_steps(*blocks[k], pts),
                    work_steps(*blocks[k - 2], pts, ps_w),
                )
            # window 7: S(B7) || work(B5) then work(B6) (exp(B6) is already
            # done by the time PE reaches it -- ScalarE runs a block behind)
            interleave(
                s_steps(*blocks[7], pts),
                chain(work_steps(*blocks[5], pts, ps_w),
                      work_steps(*blocks[6], pts, ps_w)),
            )
            # tail: only the last block's PV/norm/proj remains; its y
            # copies go to ScalarE, idle after the final exp (B7's stores
            # can't start earlier anyway, and this unloads the DVE queue)
            drain(work_steps(*blocks[7], pts, ps_w, y_eng=nc.scalar))

    nc.compile()
    return nc


def make_core_inputs(x, Wqkv, Wproj):
    """Per-core input dicts. Core c: batch c//4, heads 4*(c%4) .. 4*(c%4)+3."""
    bf16 = mybir.dt.np(BF16)
    scale = D**-0.5  # 1/8, exact in bf16
    xts = [np.ascontiguousarray(x[b].T).astype(bf16) for b in range(B)]
    in_maps = []
    for core in range(8):
        b, hg = core // 4, core % 4
        heads = [HPC * hg + i for i in range(HPC)]
        rows_q = np.concatenate([Wqkv[D * h : D * (h + 1)] for h in heads]) * scale
        rows_k = np.concatenate([Wqkv[C + D * h : C + D * (h + 1)] for h in heads])
        wqk = np.ascontiguousarray(np.concatenate([rows_q, rows_k]).T).astype(bf16)
        wv = np.ascontiguousarray(
            np.concatenate([Wqkv[2 * C + D * h : 2 * C + D * (h + 1)] for h in heads]).T
        ).astype(bf16)
        wp = np.ascontiguousarray(
            np.concatenate([Wproj[:, D * h : D * (h + 1)] for h in heads], axis=1).T
        ).astype(bf16)
        in_maps.append({"xt": xts[b], "wqk": wqk, "wv": wv, "wp": wp})
    return in_maps


_EXEC_CACHE = {}


def _get_executor():
    """Build + jit the 8-core SPMD executable once per process."""
    if "fn" in _EXEC_CACHE:
        return _EXEC_CACHE
    import jax
    from jax.sharding import Mesh, PartitionSpec
    from jax.experimental.shard_map import shard_map
    from concourse import bass2jax
    from concourse.bass2jax import _bass_exec_p, partition_id_tensor

    nc = build_bass()
    bass2jax.install_neuronx_cc_hook()
    pid = nc.partition_id_tensor.name if nc.partition_id_tensor else None
    in_names, out_names, out_avals = [], [], []
    for alloc in nc.m.functions[0].allocations:
        if not isinstance(alloc, mybir.MemoryLocationSet):
            continue
        name = alloc.memorylocations[0].name
        if alloc.kind == "ExternalInput":
            if name != pid:
                in_names.append(name)
        elif alloc.kind == "ExternalOutput":
            out_names.append(name)
            out_avals.append(
                jax.core.ShapedArray(
                    tuple(alloc.tensor_shape), mybir.dt.np(alloc.dtype)
                )
            )
    n_params = len(in_names)
    all_names = list(in_names) + list(out_names) + ([pid] if pid else [])

    def body(*args):
        *ins, yb = args
        operands = list(ins) + [yb]
        if pid:
            operands.append(partition_id_tensor())
        outs = _bass_exec_p.bind(
            *operands,
            out_avals=tuple(out_avals),
            in_names=tuple(all_names),
            out_names=tuple(out_names),
            lowering_input_output_aliases=(),
            sim_require_finite=True,
            sim_require_nnan=True,
            nc=nc,
        )
        return outs[0]

    mesh = Mesh(np.asarray(jax.devices()[:8]), ("core",))
    fn = jax.jit(
        shard_map(
            body,
            mesh=mesh,
            in_specs=(PartitionSpec("core"),) * (n_params + 1),
            out_specs=PartitionSpec("core"),
            check_rep=False,
        ),
        donate_argnums=(n_params,),
    )
    _EXEC_CACHE.update(fn=fn, in_names=in_names)
    return _EXEC_CACHE


def kernel(x, Wqkv, Wproj, bproj):
    x = np.asarray(x, dtype=np.float32)
    Wqkv = np.asarray(Wqkv, dtype=np.float32)
    Wproj = np.asarray(Wproj, dtype=np.float32)
    bproj = np.asarray(bproj, dtype=np.float32)

    ex = _get_executor()
    in_maps = make_core_inputs(x, Wqkv, Wproj)
    glob_ins = [
        np.concatenate([np.asarray(m[name]) for m in in_maps], axis=0)
        for name in ex["in_names"]
    ]
    y0 = np.zeros((8 * S, C), mybir.dt.np(BF16))
    out = np.asarray(ex["fn"](*glob_ins, y0))  # [8*S, C] bf16 partials

    y = np.zeros((B, S, C), dtype=np.float32)
    for core in range(8):
        y[core // 4] += np.asarray(out[core * S : (core + 1) * S, :], np.float32)
    y += bproj
    return y


# revision 26
# speedup vs baseline: 1.0521x; 1.0137x over previous
"""Multi-head attention block (B=2, N=2048, C=1024, H=16, D=64) on 8 TRN2 cores.

Sharding: core c -> batch b = c // 4, head-group hg = c % 4 (4 heads per core).
All matmuls in bf16 (1 cyc/row at any moving size in the cost model).
Per core:
  qkT  = Wqk_hg @ x_b^T          (f-tiles [2 heads x 64d, 2048 tok], bf16)
  V    = x_b @ Wv_hg^T           ([tok, 4h x 64d] natural layout -> vaug [V|1])
  S^T  = kT^T q-chunks           (K=64, two heads row-packed; [128 key, 512 q])
  P^T  = exp(S^T)                (ScalarE, no max-subtraction: scores ~ N(0,1))
  O|Z  = P^T-tile^T @ [V|1]      (q-major: out [128 q, 65]; V is the moving
                                  operand -> 65 rows/pass instead of 512)
  O_n  = O * (1/Z)               (DVE reciprocal + per-partition tensor_scalar)
  O^T  = DMA-transpose(O_n)      (XBAR, SBUF->SBUF, 2-byte)
  y    = O^T^T @ Wp^T            (K=128: two heads stacked per pass)
Host sums the 4 head-group partials per batch and adds bias.

Issue order is software-pipelined: S-matmul groups for block k+1 are
interleaved with PV/normalize/proj work of block k-2 so the ScalarE exp
stream (the bottleneck engine, ~133us) never starves.
"""

import numpy as np

import concourse.bass as bass
import concourse.tile as tile
from concourse import bacc, mybir

F32 = mybir.dt.float32
BF16 = mybir.dt.bfloat16
EXP = mybir.ActivationFunctionType.Exp

B, S, C = 2, 2048, 1024
H, D = 16, 64
HPC = 4            # heads per core
NCT = C // 128     # 8 contraction tiles
MT = S // 128      # 16 key tiles
NCH = S // 512     # 4 query chunks


def build_bass(loop_n=None):
    nc = bacc.Bacc("TRN2", target_bir_lowering=False)

    xt_d = nc.dram_tensor("xt", [C, S], BF16, kind="ExternalInput")
    wqk_d = nc.dram_tensor("wqk", [C, 512], BF16, kind="ExternalInput")
    wv_d = nc.dram_tensor("wv", [C, 256], BF16, kind="ExternalInput")
    wp_d = nc.dram_tensor("wp", [256, C], BF16, kind="ExternalInput")
    y_d = nc.dram_tensor("y", [S, C], BF16, kind="ExternalOutput")

    def mm(out, lhsT, rhs, start, stop):
        nc.tensor.matmul(out, lhsT, rhs, start=start, stop=stop)

    with tile.TileContext(nc) as tc:
        with (
            tc.tile_pool(name="persist", bufs=1) as persist,
            tc.tile_pool(name="pt", bufs=6) as pt_pool,
            tc.tile_pool(name="rz", bufs=4) as rz_pool,
            tc.tile_pool(name="osb", bufs=8) as osb_pool,
            tc.tile_pool(name="ysb", bufs=4) as ysb_pool,
            tc.tile_pool(name="ps_s", bufs=2, space="PSUM") as ps_s,
            tc.tile_pool(name="ps_w", bufs=4, space="PSUM") as ps_w,
        ):
            # ---- persistent SBUF tiles ----
            # qkT f-tiles: 0=q_h0|q_h1, 1=q_h2|q_h3, 2=k_h0|k_h1, 3=k_h2|k_h3
            qk_sb = persist.tile([128, 4 * S], BF16, tag="qk")
            # V augmented per key tile m: [v_h0|1|v_h1|1|v_h2|1|v_h3|1]
            vaug = persist.tile([128, MT * 260], BF16, tag="vaug")
            wp_sb = persist.tile([128, 2 * C], BF16, tag="wp")
            xt_sb = persist.tile([128, NCT * S], BF16, tag="xt")
            wqk_sb = persist.tile([128, NCT * 512], BF16, tag="wqk")
            wv_sb = persist.tile([128, NCT * 256], BF16, tag="wv")
            # normalized-transposed O per (pair, ch): [128 = 2 heads x 64d, 512 q]
            onorm = [
                persist.tile([64, 0], BF16, tag="dummy")  # placeholder, replaced below
                for _ in range(0)
            ]
            onorm = {}
            for p in range(2):
                for c in range(NCH):
                    onorm[(p, c)] = persist.tile(
                        [128, 512], BF16, tag=f"onorm{p}{c}", name=f"onorm{p}{c}"
                    )

            nc.vector.memset(vaug, 1.0)

            def dma_wqk_f(f):
                # wqk_sb layout: [128, f*1024 + ct*128 + j]
                nc.sync.dma_start(
                    out=wqk_sb[:, f * 1024 : (f + 1) * 1024].rearrange(
                        "p (c j) -> p c j", c=NCT
                    ),
                    in_=wqk_d[:, f * 128 : (f + 1) * 128].rearrange(
                        "(c p) j -> p c j", p=128
                    ),
                )

            # DMA priority order: weights the first S block needs, then x,
            # then V weights (window-0 fillers), then the pair-1 weights.
            dma_wqk_f(2)
            dma_wqk_f(0)
            for ct in range(NCT):
                nc.sync.dma_start(
                    out=xt_sb[:, ct * S : (ct + 1) * S],
                    in_=xt_d[ct * 128 : (ct + 1) * 128, :],
                )
            for ct in range(NCT):
                nc.sync.dma_start(
                    out=wv_sb[:, ct * 256 : (ct + 1) * 256],
                    in_=wv_d[ct * 128 : (ct + 1) * 128, :],
                )
            for f in (3, 1):
                dma_wqk_f(f)
            for p in range(2):
                nc.sync.dma_start(
                    out=wp_sb[:, p * C : (p + 1) * C],
                    in_=wp_d[p * 128 : (p + 1) * 128, :],
                )

            # ---------------- step generators ----------------

            def qk_f_block(chains, ps_a):
                """qkT chains [(f, sc), ...] ct-outer concurrently (needs
                len(chains) free ps_a bufs). Used during the input DMA
                stream so PE accumulates as each x chunk lands."""
                qps = {
                    fs: ps_a.tile([128, 512], F32, tag="psw",
                                  name=f"qps{fs[0]}_{fs[1]}")
                    for fs in chains
                }
                for ct in range(NCT):
                    for f, sc in chains:
                        mm(
                            qps[(f, sc)],
                            wqk_sb[:, f * 1024 + ct * 128 : f * 1024 + (ct + 1) * 128],
                            xt_sb[:, ct * S + sc * 512 : ct * S + (sc + 1) * 512],
                            start=(ct == 0),
                            stop=(ct == NCT - 1),
                        )
                for f, sc in chains:
                    nc.vector.tensor_copy(
                        qk_sb[:, f * S + sc * 512 : f * S + (sc + 1) * 512],
                        qps[(f, sc)],
                    )

            def qk_f_steps(f, ps_a, scs=range(4)):
                """Half-sc ct-inner steps (~0.85us each) for filler use once
                the input DMAs have landed."""
                for sc in scs:
                    qps_box = []
                    def step1(f=f, sc=sc, qps_box=qps_box):
                        qps = ps_a.tile([128, 512], F32, tag="psw",
                                        name=f"qps{f}_{sc}")
                        qps_box.append(qps)
                        for ct in range(NCT // 2):
                            mm(
                                qps,
                                wqk_sb[:, f * 1024 + ct * 128 : f * 1024 + (ct + 1) * 128],
                                xt_sb[:, ct * S + sc * 512 : ct * S + (sc + 1) * 512],
                                start=(ct == 0),
                                stop=False,
                            )
                    def step2(f=f, sc=sc, qps_box=qps_box):
                        qps = qps_box[0]
                        for ct in range(NCT // 2, NCT):
                            mm(
                                qps,
                                wqk_sb[:, f * 1024 + ct * 128 : f * 1024 + (ct + 1) * 128],
                                xt_sb[:, ct * S + sc * 512 : ct * S + (sc + 1) * 512],
                                start=False,
                                stop=(ct == NCT - 1),
                            )
                        nc.vector.tensor_copy(
                            qk_sb[:, f * S + sc * 512 : f * S + (sc + 1) * 512],
                            qps,
                        )
                    yield step1
                    yield step2

            def v_steps(m0, m1, pool, tag):
                """V key-tiles m0..m1-1 -> vaug (strided, ones preserved)."""
                for m in range(m0, m1):
                    def step(m=m, pool=pool, tag=tag):
                        vps_full = pool.tile([128, 512], F32, tag=tag,
                                             name=f"vps{m}")
                        vps = vps_full[:, 0:256]
                        for ct in range(NCT):
                            mm(
                                vps,
                                xt_sb[:, ct * S + m * 128 : ct * S + (m + 1) * 128],
                                wv_sb[:, ct * 256 : (ct + 1) * 256],
                                start=(ct == 0),
                                stop=(ct == NCT - 1),
                            )
                        nc.vector.tensor_copy(
                            vaug[:, m * 260 : (m + 1) * 260].rearrange(
                                "p (h c) -> p h c", c=65
                            )[:, :, 0:64],
                            vps.rearrange("p (h c) -> p h c", c=64),
                        )
                    yield step

            def s_steps(pair, ch, pts, head_major=False):
                """Score+exp stream for block (pair, ch). Default: 8 mg
                steps, each head A then head B over key tiles 2mg, 2mg+1.
                head_major: 16 single-head steps, all of head A first (lets
                the tail start head-A PV while head-B exps still run)."""
                qf, kf = pair, 2 + pair
                ptA = pt_pool.tile([128, MT * 512], BF16, tag="pt",
                                   name=f"ptA_{pair}_{ch}")
                ptB = pt_pool.tile([128, MT * 512], BF16, tag="pt",
                                   name=f"ptB_{pair}_{ch}")
                pts[(pair, ch)] = (ptA, ptB)

                def sub(mg, hh, pt):
                    lo = 64 * hh
                    sps = ps_s.tile([128, 1024], F32, tag="sps",
                                    name=f"s{pair}{ch}{mg}{hh}")
                    for j in range(2):
                        m = 2 * mg + j
                        mm(
                            sps[:, j * 512 : (j + 1) * 512],
                            qk_sb[lo:lo + 64, kf * S + m * 128 : kf * S + (m + 1) * 128],
                            qk_sb[lo:lo + 64, qf * S + ch * 512 : qf * S + (ch + 1) * 512],
                            start=True,
                            stop=True,
                        )
                    nc.scalar.activation(
                        pt[:, 2 * mg * 512 : (2 * mg + 2) * 512], sps, EXP
                    )

                if head_major:
                    for hh, pt in ((0, ptA), (1, ptB)):
                        for mg in range(MT // 2):
                            def step(mg=mg, hh=hh, pt=pt):
                                sub(mg, hh, pt)
                            yield step
                else:
                    for mg in range(MT // 2):
                        def step(mg=mg, ptA=ptA, ptB=ptB):
                            sub(mg, 0, ptA)
                            sub(mg, 1, ptB)
                        yield step

            osbs = {}

            def work_steps(pair, ch, pts, ps_w, phases="AB", y_eng=None):
                """PV + normalize + transpose for block (pair, ch); when
                pair==1 also project+store. phases: "AB" both heads,
                "A"/"B" only that head (B also transposes/projects).
                y_eng: engine for the y PSUM->SBUF copy (default DVE)."""
                ptA, ptB = pts[(pair, ch)]
                for qt in range(4):
                    for hh, pt in ((0, ptA), (1, ptB)):
                        if "AB"[hh] not in phases:
                            continue
                        def hstep(pair=pair, ch=ch, qt=qt, hh=hh, pt=pt):
                            if hh == 0:
                                osbs[(pair, ch, qt)] = osb_pool.tile(
                                    [128, 128], BF16, tag="osb",
                                    name=f"osb{pair}{ch}{qt}")
                            osb = osbs[(pair, ch, qt)]
                            h = 2 * pair + hh
                            ops = ps_w.tile([128, 512], F32, tag="psw",
                                            name=f"o{pair}{ch}{qt}{hh}")
                            for m in range(MT):
                                mm(
                                    ops[:, 0:65],
                                    pt[:, m * 512 + qt * 128 : m * 512 + qt * 128 + 128],
                                    vaug[:, m * 260 + h * 65 : m * 260 + h * 65 + 65],
                                    start=(m == 0),
                                    stop=(m == MT - 1),
                                )
                            rz = rz_pool.tile([128, 1], F32, tag="rz",
                                              name=f"rz{pair}{ch}{qt}{hh}")
                            nc.vector.reciprocal(out=rz, in_=ops[:, 64:65])
                            nc.vector.tensor_scalar_mul(
                                osb[:, hh * 64 : hh * 64 + 64],
                                ops[:, 0:64],
                                rz,
                            )
                            if hh == 1:
                                # full [128 q, 128 d-pair] -> [128 d-pair, 128 q]
                                nc.sync.dma_start(
                                    out=onorm[(pair, ch)][:, qt * 128 : (qt + 1) * 128],
                                    in_=osb,
                                    transpose=True,
                                )
                        yield hstep
                    if pair == 1 and "B" in phases:
                        for fc in range(2):
                            def pstep(ch=ch, qt=qt, fc=fc):
                                st = ch * 4 + qt
                                ysb = ysb_pool.tile([128, 512], BF16,
                                                    tag="ysb",
                                                    name=f"ysb{ch}{qt}{fc}")
                                yps = ps_w.tile([128, 512], F32, tag="psw",
                                                name=f"y{ch}{qt}{fc}")
                                for p in range(2):
                                    mm(
                                        yps,
                                        onorm[(p, ch)][:, qt * 128 : (qt + 1) * 128],
                                        wp_sb[:, p * C + fc * 512 : p * C + (fc + 1) * 512],
                                        start=(p == 0),
                                        stop=(p == 1),
                                    )
                                if y_eng is None:
                                    nc.vector.tensor_copy(ysb, yps)
                                else:
                                    y_eng.copy(ysb, yps)
                                nc.sync.dma_start(
                                    out=y_d[st * 128 : (st + 1) * 128,
                                            fc * 512 : (fc + 1) * 512],
                                    in_=ysb,
                                )
                            yield pstep

            def s_steps_h(half, pts):
                """Last block (p1,c3) split into 256-query halves: slot
                layout col = (2m+hh)*256, exp covers 4 slots (1024 cols)."""
                pair, ch = 1, 3
                qf, kf = pair, 2 + pair
                q0 = ch * 512 + half * 256
                pth = pt_pool.tile([128, MT * 512], BF16, tag="pt",
                                   name=f"pth{half}")
                pts[("h", half)] = pth
                for mg in range(8):
                    def step(mg=mg, pth=pth, q0=q0, kf=kf, qf=qf):
                        sps = ps_s.tile([128, 1024], F32, tag="sps",
                                        name=f"sh{mg}")
                        for i in range(4):
                            m, hh = 2 * mg + i // 2, i % 2
                            lo = 64 * hh
                            mm(
                                sps[:, i * 256 : (i + 1) * 256],
                                qk_sb[lo:lo + 64, kf * S + m * 128 : kf * S + (m + 1) * 128],
                                qk_sb[lo:lo + 64, qf * S + q0 : qf * S + q0 + 256],
                                start=True,
                                stop=True,
                            )
                        nc.scalar.activation(
                            pth[:, mg * 1024 : (mg + 1) * 1024], sps, EXP
                        )
                    yield step

            def work_steps_h(half, pts, y_eng=None):
                pair, ch = 1, 3
                pth = pts[("h", half)]
                for qtl in range(2):
                    qt = 2 * half + qtl
                    for hh in (0, 1):
                        def hstep(qt=qt, qtl=qtl, hh=hh, pth=pth):
                            if hh == 0:
                                osbs[("h", qt)] = osb_pool.tile(
                                    [128, 128], BF16, tag="osb",
                                    name=f"osbh{qt}")
                            osb = osbs[("h", qt)]
                            h = 2 * pair + hh
                            ops = ps_w.tile([128, 512], F32, tag="psw",
                                            name=f"oh{qt}{hh}")
                            for m in range(MT):
                                s0 = (2 * m + hh) * 256
                                mm(
                                    ops[:, 0:65],
                                    pth[:, s0 + qtl * 128 : s0 + qtl * 128 + 128],
                                    vaug[:, m * 260 + h * 65 : m * 260 + h * 65 + 65],
                                    start=(m == 0),
                                    stop=(m == MT - 1),
                                )
                            rz = rz_pool.tile([128, 1], F32, tag="rz",
                                              name=f"rzh{qt}{hh}")
                            nc.vector.reciprocal(out=rz, in_=ops[:, 64:65])
                            nc.vector.tensor_scalar_mul(
                                osb[:, hh * 64 : hh * 64 + 64],
                                ops[:, 0:64],
                                rz,
                            )
                            if hh == 1:
                                nc.sync.dma_start(
                                    out=onorm[(1, 3)][:, qt * 128 : (qt + 1) * 128],
                                    in_=osb,
                                    transpose=True,
                                )
                        yield hstep
                    for fc in range(2):
                        def pstep(qt=qt, fc=fc, y_eng=y_eng):
                            st = 3 * 4 + qt
                            ysb = ysb_pool.tile([128, 512], BF16, tag="ysb",
                                                name=f"ysbh{qt}{fc}")
                            yps = ps_w.tile([128, 512], F32, tag="psw",
                                            name=f"yh{qt}{fc}")
                            for p in range(2):
                                mm(
                                    yps,
                                    onorm[(p, 3)][:, qt * 128 : (qt + 1) * 128],
                                    wp_sb[:, p * C + fc * 512 : p * C + (fc + 1) * 512],
                                    start=(p == 0),
                                    stop=(p == 1),
                                )
                            if y_eng is None:
                                nc.vector.tensor_copy(ysb, yps)
                            else:
                                y_eng.copy(ysb, yps)
                            nc.sync.dma_start(
                                out=y_d[st * 128 : (st + 1) * 128,
                                        fc * 512 : (fc + 1) * 512],
                                in_=ysb,
                            )
                        yield pstep

            def chain(*gens):
                for g in gens:
                    yield from g

            def interleave(sgen, fgen, s_head=0):
                """Alternate one S step with one filler step; drain leftovers.
                s_head: emit this many S steps before the first filler."""
                for _ in range(s_head):
                    s = next(sgen, None)
                    if s is not None:
                        s()
                while True:
                    s = next(sgen, None)
                    if s is not None:
                        s()
                    f = next(fgen, None)
                    if f is not None:
                        f()
                    if s is None and f is None:
                        return

            def drain(g):
                for step in g:
                    step()

            # ---------------- pipelined schedule ----------------
            blocks = [(p, c) for p in range(2) for c in range(NCH)]
            pts = {}

            # during the input DMA stream: the columns S(p0,c0) needs first
            qk_f_block([(2, 0), (0, 0), (2, 1), (2, 2)], ps_w)
            # window 0: S(p0,c0) || k0 tail + q0 rest + V m0..4
            interleave(
                s_steps(0, 0, pts),
                chain(
                    qk_f_steps(2, ps_w, scs=(3,)),
                    qk_f_steps(0, ps_w, scs=(1, 2, 3)),
                    v_steps(0, 5, ps_w, 'psw'),
                ),
                s_head=3,
            )
            # window 1: S(p0,c1) || V m5..15
            interleave(
                s_steps(0, 1, pts),
                v_steps(5, MT, ps_w, 'psw'),
            )
            # window 2: S(p0,c2) || k1 weights + work(B0)
            interleave(
                s_steps(0, 2, pts),
                chain(qk_f_steps(3, ps_w),
                      work_steps(*blocks[0], pts, ps_w)),
            )
            # window 3: S(p0,c3) || q1 weights + work(B1)
            interleave(
                s_steps(0, 3, pts),
                chain(qk_f_steps(1, ps_w),
                      work_steps(*blocks[1], pts, ps_w)),
            )
            # windows 4..6: S(block k) || work(block k-2)
            for k in range(4, len(blocks) - 1):
                interleave(
                    s_steps(*blocks[k], pts),
                    work_steps(*blocks[k - 2], pts, ps_w),
                )
            # window 7a: first 256-query half of the last block || work(B5),
            # work(B6) (exp(B6) done by the time PE reaches it)
            interleave(
                s_steps_h(0, pts),
                chain(work_steps(*blocks[5], pts, ps_w),
                      work_steps(*blocks[6], pts, ps_w)),
            )
            # window 7b: second half || first half's PV/norm/proj (its exps
            # are done while the second half's still stream)
            interleave(
                s_steps_h(1, pts),
                work_steps_h(0, pts),
            )
            # tail: only half a block remains; ScalarE (idle) takes y copies
            drain(work_steps_h(1, pts, y_eng=nc.scalar))

    nc.compile()
    return nc


def make_core_inputs(x, Wqkv, Wproj):
    """Per-core input dicts. Core c: batch c//4, heads 4*(c%4) .. 4*(c%4)+3."""
    bf16 = mybir.dt.np(BF16)
    scale = D**-0.5  # 1/8, exact in bf16
    xts = [np.ascontiguousarray(x[b].T).astype(bf16) for b in range(B)]
    in_maps = []
    for core in range(8):
        b, hg = core // 4, core % 4
        heads = [HPC * hg + i for i in range(HPC)]
        rows_q = np.concatenate([Wqkv[D * h : D * (h + 1)] for h in heads]) * scale
        rows_k = np.concatenate([Wqkv[C + D * h : C + D * (h + 1)] for h in heads])
        wqk = np.ascontiguousarray(np.concatenate([rows_q, rows_k]).T).astype(bf16)
        wv = np.ascontiguousarray(
            np.concatenate([Wqkv[2 * C + D * h : 2 * C + D * (h + 1)] for h in heads]).T
        ).astype(bf16)
        wp = np.ascontiguousarray(
            np.concatenate([Wproj[:, D * h : D * (h + 1)] for h in heads], axis=1).T
        ).astype(bf16)
        in_maps.append({"xt": xts[b], "wqk": wqk, "wv": wv, "wp": wp})
    return in_maps


_EXEC_CACHE = {}


def _get_executor():
    """Build + jit the 8-core SPMD executable once per process."""
    if "fn" in _EXEC_CACHE:
        return _EXEC_CACHE
    import jax
    from jax.sharding import Mesh, PartitionSpec
    from jax.experimental.shard_map import shard_map
    from concourse import bass2jax
    from concourse.bass2jax import _bass_exec_p, partition_id_tensor

    nc = build_bass()
    bass2jax.install_neuronx_cc_hook()
    pid = nc.partition_id_tensor.name if nc.partition_id_tensor else None
    in_names, out_names, out_avals = [], [], []
    for alloc in nc.m.functions[0].allocations:
        if not isinstance(alloc, mybir.MemoryLocationSet):
            continue
        name = alloc.memorylocations[0].name
        if alloc.kind == "ExternalInput":
            if name != pid:
                in_names.append(name)
        elif alloc.kind == "ExternalOutput":
            out_names.append(name)
            out_avals.append(
                jax.core.ShapedArray(
                    tuple(alloc.tensor_shape), mybir.dt.np(alloc.dtype)
                )
            )
    n_params = len(in_names)
    all_names = list(in_names) + list(out_names) + ([pid] if pid else [])

    def body(*args):
        *ins, yb = args
        operands = list(ins) + [yb]
        if pid:
            operands.append(partition_id_tensor())
        outs = _bass_exec_p.bind(
            *operands,
            out_avals=tuple(out_avals),
            in_names=tuple(all_names),
            out_names=tuple(out_names),
            lowering_input_output_aliases=(),
            sim_require_finite=True,
            sim_require_nnan=True,
            nc=nc,
        )
        return outs[0]

    mesh = Mesh(np.asarray(jax.devices()[:8]), ("core",))
    fn = jax.jit(
        shard_map(
            body,
            mesh=mesh,
            in_specs=(PartitionSpec("core"),) * (n_params + 1),
            out_specs=PartitionSpec("core"),
            check_rep=False,
        ),
        donate_argnums=(n_params,),
    )
    _EXEC_CACHE.update(fn=fn, in_names=in_names)
    return _EXEC_CACHE


def kernel(x, Wqkv, Wproj, bproj):
    x = np.asarray(x, dtype=np.float32)
    Wqkv = np.asarray(Wqkv, dtype=np.float32)
    Wproj = np.asarray(Wproj, dtype=np.float32)
    bproj = np.asarray(bproj, dtype=np.float32)

    ex = _get_executor()
    in_maps = make_core_inputs(x, Wqkv, Wproj)
    glob_ins = [
        np.concatenate([np.asarray(m[name]) for m in in_maps], axis=0)
        for name in ex["in_names"]
    ]
    y0 = np.zeros((8 * S, C), mybir.dt.np(BF16))
    out = np.asarray(ex["fn"](*glob_ins, y0))  # [8*S, C] bf16 partials

    y = np.zeros((B, S, C), dtype=np.float32)
    for core in range(8):
        y[core // 4] += np.asarray(out[core * S : (core + 1) * S, :], np.float32)
    y += bproj
    return y
